# revision 52
# baseline (speedup 1.0000x reference)
"""BERT multi-head self-attention on 8 Trainium2 NeuronCores.

Problem: B=2, S=2048, H=768, NH=12, HD=64 (fp32 reference).

Sharding (hardcoded): core c in 0..7 handles batch b=c//4 and head group
g=c%4 (heads 3g..3g+2).  Each core computes its 3 heads' attention plus the
partial output projection ctx_g @ Wo[rows of g]; the host sums the 4 partial
outputs per batch element and adds the (bv @ Wo + bo) constant row.

Device pipeline per core (matmuls on PE, exp on ACT, evictions on DVE):
  1. QKV projections as fp8e4 DoubleRow matmuls: x and the (host-side
     64x-scaled) weights are split hi/lo into two e4m3 planes; the three
     cross terms xh@wh + xl@wh + xh@wl accumulate 256-deep contractions per
     instruction (25% fewer PE cycles than bf16) at bf16-grade accuracy.
     The PSUM evictions rescale by 1/64 and add the bias.
  2. V natural [seq, head_dim] with a ones-augmented column per head, so the
     P@V_aug matmul yields both ctx^T and the softmax denominator.
  3. scoresT[k, q] = K^T.T @ Q^T per 128-row k-block; ACT computes
     exp(0.125*scores) straight out of PSUM.
  4. ctxT_aug[d+1, q] accumulates over k-blocks in PSUM; row 64 is the
     denominator.  Reciprocal, then a partition-broadcast of 1/denom (bounced
     through a DRAM scratch row) and a DVE multiply normalize ctx^T.
  5. out[q, hout] = ctxT_norm.T @ Wo_slice per 128-row q-block; evictions
     stay off the ACT engine so it runs nothing but the exp stream.

Scheduling: one flat software pipeline over all (head, q-half, k-block)
units in which scores/exp lead their P@V consumer.  Only a 5-unit projection
prefix runs before the first exp; the remaining QKV-projection work drains
one unit per iteration inside blocks 0-1, in deadline order, borrowing
scores-PSUM slots.  PSUM: 3 score buffers x 2 banks + ctx accumulator x 2.
"""

import os
import sys
import numpy as np

for _p in ("/opt/trn_rl_repo",):
    if _p not in sys.path and os.path.isdir(_p):
        sys.path.append(_p)

import ml_dtypes  # noqa: E402

from concourse import bacc  # noqa: E402
import concourse.mybir as mybir  # noqa: E402
import concourse.tile as tile  # noqa: E402
from concourse.bass_utils import run_bass_kernel_spmd  # noqa: E402

B, S, H = 2, 2048, 768
NH, HD = 12, 64
HPC = 3
NCORES = 8
P = 128
NKB = S // P
NQB = S // P
NHC = H // P
NCP = NHC // 2
QH = 1024
NQH = S // QH
F32 = mybir.dt.float32
F8 = mybir.dt.float8e4

CDT = mybir.dt.bfloat16
NP_CDT = ml_dtypes.bfloat16
NP_F8 = ml_dtypes.float8_e4m3

WS = 64.0
DR = mybir.MatmulPerfMode.DoubleRow


def _build_nc(use_mask: bool):
    import contextlib

    nc = bacc.Bacc("TRN2", target_bir_lowering=False)
    AF = mybir.ActivationFunctionType
    MULT, ADD = mybir.AluOpType.mult, mybir.AluOpType.add

    xt8h = nc.dram_tensor("xt8h", [H, S], F8, kind="ExternalInput")
    xt8l = nc.dram_tensor("xt8l", [H, S], F8, kind="ExternalInput")
    wqk8h = nc.dram_tensor("wqk8h", [H, 2 * HPC * HD], F8, kind="ExternalInput")
    wqk8l = nc.dram_tensor("wqk8l", [H, 2 * HPC * HD], F8, kind="ExternalInput")
    wv8h = nc.dram_tensor("wv8h", [H, HPC * HD], F8, kind="ExternalInput")
    wv8l = nc.dram_tensor("wv8l", [H, HPC * HD], F8, kind="ExternalInput")
    wo = nc.dram_tensor("wo", [HPC * HD, H], CDT, kind="ExternalInput")
    bqk = nc.dram_tensor("bqk", [2 * HPC * HD, 1], F32, kind="ExternalInput")
    if use_mask:
        mv = nc.dram_tensor("mv", [S, 1], F32, kind="ExternalInput")
    out = nc.dram_tensor("out", [S, H], CDT, kind="ExternalOutput")
    rspill = nc.dram_tensor("rspill", [HPC * NQH, QH], F32)

    with tile.TileContext(nc) as tc, contextlib.ExitStack() as ctx, \
            nc.allow_low_precision(reason="fp8/bf16 compute pipeline by design"):
        const = ctx.enter_context(tc.tile_pool(name="const", bufs=1))
        xt_pool = ctx.enter_context(tc.tile_pool(name="xt", bufs=1))
        w_pool = ctx.enter_context(tc.tile_pool(name="w", bufs=1))
        qkt_pool = ctx.enter_context(tc.tile_pool(name="qkt", bufs=1))
        v_pool = ctx.enter_context(tc.tile_pool(name="v", bufs=1))
        pt_pool = ctx.enter_context(tc.tile_pool(name="pt", bufs=6))
        ctxu_pool = ctx.enter_context(tc.tile_pool(name="ctxu", bufs=1))
        ctxn_pool = ctx.enter_context(tc.tile_pool(name="ctxn", bufs=1))
        out_sb_pool = ctx.enter_context(tc.tile_pool(name="outsb", bufs=6))

        bias_t = const.tile([P, 3], F32, tag="bqk")
        nc.sync.dma_start(
            bias_t[:], bqk[:].rearrange("(m p) one -> p (m one)", p=P))
        wqk8h_t = w_pool.tile([P, NHC, 2 * HPC * HD], F8, tag="wqk8h")
        wqk8l_t = w_pool.tile([P, NHC, 2 * HPC * HD], F8, tag="wqk8l")
        wv8h_t = w_pool.tile([P, NHC, HPC * HD], F8, tag="wv8h")
        wv8l_t = w_pool.tile([P, NHC, HPC * HD], F8, tag="wv8l")
        nc.sync.dma_start(
            wqk8h_t[:], wqk8h[:].rearrange("(c p) n -> p c n", p=P))
        nc.scalar.dma_start(
            wqk8l_t[:], wqk8l[:].rearrange("(c p) n -> p c n", p=P))
        nc.scalar.dma_start(
            wv8h_t[:], wv8h[:].rearrange("(c p) n -> p c n", p=P))
        nc.scalar.dma_start(
            wv8l_t[:], wv8l[:].rearrange("(c p) n -> p c n", p=P))
        xt8h_p = [xt_pool.tile([P, 2, S], F8, tag=f"xt8h{cp}",
                               name=f"xt8h{cp}") for cp in range(NCP)]
        xt8l_p = [xt_pool.tile([P, 2, S], F8, tag=f"xt8l{cp}",
                               name=f"xt8l{cp}") for cp in range(NCP)]

        def xt_piece(eng, dst_p, src, cp, qs):
            eng.dma_start(
                dst_p[cp][:, :, qs],
                src[2 * cp * P:(2 * cp + 2) * P, qs].rearrange(
                    "(c p) n -> p c n", p=P))

        for qg in range(2):
            qs = slice(qg * QH, (qg + 1) * QH)
            for cp in range(NCP):
                xt_piece(nc.sync, xt8h_p, xt8h, cp, qs)
                xt_piece(nc.scalar, xt8l_p, xt8l, cp, qs)
        bias_sb = [bias_t[:, m:m + 1] for m in range(3)]
        if use_mask:
            mv_t = const.tile([P, NKB], F32, tag="mv")
            nc.scalar.dma_start(
                mv_t[:], mv[:].rearrange("(kb p) one -> p (kb one)", p=P))
            mv_sb = [mv_t[:, kb:kb + 1] for kb in range(NKB)]
        wo_t = w_pool.tile([HD, HPC, H], CDT, tag="wo")
        nc.scalar.dma_start(
            wo_t[:], wo[:].rearrange("(h p) n -> p h n", p=HD))
        wo_sb = [wo_t[:, h, :] for h in range(HPC)]

        tq01 = qkt_pool.tile([P, S], CDT, tag="tq01")
        tk01 = qkt_pool.tile([P, S], CDT, tag="tk01")
        tqk2 = qkt_pool.tile([P, S], CDT, tag="tqk2")
        qkt_tiles = [tq01, tk01, tqk2]
        v_sb = [None] * NKB

        HILO = ((xt8h_p, wqk8h_t, wv8h_t), (xt8l_p, wqk8h_t, wv8h_t),
                (xt8h_p, wqk8l_t, wv8l_t))

        def qkt_unit(psum_tile_fn, qc, m):
            qs = slice(qc * 512, (qc + 1) * 512)
            ms = slice(m * P, (m + 1) * P)
            ps = psum_tile_fn([P, 512], "qkps")
            for t, (xa, wa, _) in enumerate(HILO):
                for cp in range(NCP):
                    cs = slice(2 * cp, 2 * cp + 2)
                    nc.tensor.matmul(
                        ps[:], wa[:, cs, ms], xa[cp][:, :, qs],
                        start=(t == 0 and cp == 0),
                        stop=(t == 2 and cp == NCP - 1),
                        perf_mode=DR,
                    )
            nc.vector.tensor_scalar(
                qkt_tiles[m][:, qs], ps[:], 1.0 / WS, bias_sb[m][:],
                MULT, ADD,
            )

        def v_unit(psum_tile_fn, kb):
            ks = slice(kb * P, (kb + 1) * P)
            ps = psum_tile_fn([P, HPC * HD], "vps")
            for t, (xa, _, va) in enumerate(HILO):
                for cp in range(NCP):
                    cs = slice(2 * cp, 2 * cp + 2)
                    nc.tensor.matmul(
                        ps[:], xa[cp][:, :, ks], va[:, cs, :],
                        start=(t == 0 and cp == 0),
                        stop=(t == 2 and cp == NCP - 1),
                        perf_mode=DR,
                    )
            vt = v_pool.tile([P, HPC, HD + 1], CDT, tag=f"v{kb}",
                             name=f"vt{kb}")
            nc.vector.tensor_scalar_mul(
                vt[:, :, 0:HD], ps[:].rearrange("p (h d) -> p h d", h=HPC),
                1.0 / WS,
            )
            nc.vector.memset(vt[:, :, HD:HD + 1], 1.0)
            if use_mask:
                nc.vector.tensor_scalar_mul(vt[:], vt[:], mv_sb[kb][:])
            v_sb[kb] = vt

        with tc.tile_pool(name="qkt_psum", bufs=2, space="PSUM") as qkt_psum:
            def pre_tile(shape, name):
                return qkt_psum.tile(shape, F32, tag="qkt", name=name)
            qkt_unit(pre_tile, 0, 0)
            qkt_unit(pre_tile, 0, 1)
            qkt_unit(pre_tile, 1, 0)
            qkt_unit(pre_tile, 1, 1)
            v_unit(pre_tile, 0)
        tk2 = qkt_pool.tile([HD, S], CDT, tag="tk2")

        def q_ap(h, sl):
            if h == 0:
                return tq01[0:HD, sl]
            if h == 1:
                return tq01[HD:2 * HD, sl]
            return tqk2[0:HD, sl]

        def k_ap(h, sl):
            if h == 0:
                return tk01[0:HD, sl]
            if h == 1:
                return tk01[HD:2 * HD, sl]
            return tk2[0:HD, sl]

        ctxu_t = [ctxu_pool.tile([HD, S], F32, tag=f"ctxu{h}", name=f"ctxu{h}")
                  for h in range(HPC)]
        ctxn_t = [ctxn_pool.tile([HD, S], CDT, tag=f"ctxn{h}", name=f"ctxn{h}")
                  for h in range(HPC)]
        recip_t = [ctxu_pool.tile([65, S], F32, tag=f"recip{h}",
                                  name=f"recip{h}") for h in range(HPC)]
        rbc_pool = ctx.enter_context(tc.tile_pool(name="rbc", bufs=2))

        def op_unit(psum_tile_fn, qb):
            qsl = slice(qb * P, (qb + 1) * P)
            ops = psum_tile_fn([P, H], "ops")
            for nchunk in range(2):
                nsl = slice(nchunk * 512, min((nchunk + 1) * 512, H))
                for h in range(HPC):
                    nc.tensor.matmul(
                        ops[:, nsl],
                        ctxn_t[h][:, qsl],
                        wo_sb[h][:, nsl],
                        start=(h == 0), stop=(h == HPC - 1),
                    )
            osb = out_sb_pool.tile([P, H], CDT, tag="osb", name="osb")
            nc.vector.tensor_copy(osb[:], ops[:])
            (nc.sync if qb % 2 == 0 else nc.scalar).dma_start(
                out[qsl, :], osb[:])

        with tc.tile_pool(name="sc_psum", bufs=3, space="PSUM") as sc_psum, \
             tc.tile_pool(name="ctx_psum", bufs=1, space="PSUM") as ctx_psum:
            def sc_tile(shape, name):
                return sc_psum.tile(shape, F32, tag="sc", name=name)

            sched = ([("v", kb) for kb in (1, 2, 3, 4, 5, 6)] + [("qk", 2, 1)]
                     + [("v", kb) for kb in (7, 8, 9, 10)] + [("qk", 3, 1)]
                     + [("v", kb) for kb in (11, 12, 13, 14, 15)]
                     + [("qk", 2, 2), ("qk", 3, 2), ("qk", 0, 2),
                        ("qk", 1, 2), ("qk", 2, 0), ("qk", 3, 0)])
            fillers = []
            for u in sched:
                if u[0] == "v":
                    fillers.append(lambda kb=u[1]: v_unit(sc_tile, kb))
                else:
                    fillers.append(
                        lambda qc=u[1], m=u[2]: qkt_unit(sc_tile, qc, m))
            fillers.reverse()

            blocks = [(qh, h) for qh in range(NQH) for h in range(HPC)]

            def scores(bi, kb):
                qh, h = blocks[bi]
                ksl = slice(kb * P, (kb + 1) * P)
                sps = sc_psum.tile([P, QH], F32, tag="sc", name="sps")
                for c in range(QH // 512):
                    nc.tensor.matmul(
                        sps[:, c * 512:(c + 1) * 512],
                        k_ap(h, ksl),
                        q_ap(h, slice(qh * QH + c * 512,
                                      qh * QH + (c + 1) * 512)),
                        start=True, stop=True,
                    )
                pt = pt_pool.tile([P, QH], CDT, tag="pt", name="pt")
                nc.scalar.activation(pt[:], sps[:], AF.Exp, scale=0.125)
                return pt

            def pv(bi, kb, pt, cps):
                _, h = blocks[bi]
                for c in range(QH // 512):
                    nc.tensor.matmul(
                        cps[:, c * 512:(c + 1) * 512],
                        v_sb[kb][:, h, :],
                        pt[:, c * 512:(c + 1) * 512],
                        start=(kb == 0), stop=(kb == NKB - 1),
                    )

            def normalize(bi, cps):
                qh, h = blocks[bi]
                qsl = slice(qh * QH, (qh + 1) * QH)
                nc.vector.tensor_copy(ctxu_t[h][:, qsl], cps[0:HD, :])
                nc.vector.reciprocal(
                    recip_t[h][HD:HD + 1, qsl], cps[HD:HD + 1, :]
                )
                row = qh * HPC + h
                nc.sync.dma_start(rspill[row, :], recip_t[h][HD:HD + 1, qsl])
                rbc = rbc_pool.tile([HD, QH], F32, tag="rbc", name="rbc")
                nc.sync.dma_start(
                    rbc[:], rspill[row:row + 1, :].to_broadcast((HD, QH)))
                nc.vector.tensor_mul(ctxn_t[h][:, qsl], ctxu_t[h][:, qsl],
                                     rbc[:])

            from collections import deque
            cps_of = {}
            pending = deque()
            gi = 0

            def drain_one():
                pbi, pkb, ppt = pending.popleft()
                pv(pbi, pkb, ppt, cps_of[pbi])
                if pkb == NKB - 1:
                    normalize(pbi, cps_of[pbi])

            for bi in range(len(blocks)):
                for kb in range(NKB):
                    if kb == 0:
                        cps_of[bi] = ctx_psum.tile(
                            [HD + 1, QH], F32, tag="ctx", name=f"cps{bi}")
                        if bi == 2:
                            nc.sync.dma_start(tk2[:], tqk2[HD:2 * HD, :])
                    pending.append((bi, kb, scores(bi, kb)))
                    gi += 1
                    for _ in range(2 if gi <= 2 else 1):
                        if fillers:
                            fillers.pop()()
                    lag = 1 if fillers else 3
                    while len(pending) > lag:
                        drain_one()
            while pending:
                drain_one()

        with tc.tile_pool(name="op_psum", bufs=4, space="PSUM") as op_psum:
            def op_tile(shape, name):
                return op_psum.tile(shape, F32, tag="op", name=name)
            for qb in range(NQB):
                op_unit(op_tile, qb)

    nc.compile()
    return nc


_NC_CACHE = {}


def _get_nc(use_mask: bool):
    if use_mask not in _NC_CACHE:
        _NC_CACHE[use_mask] = _build_nc(use_mask)
    return _NC_CACHE[use_mask]


def _hilo(a):
    hi = a.astype(NP_F8)
    lo = (a - hi.astype(np.float32)).astype(NP_F8)
    return hi, lo


def _shard_inputs(hidden_states, attention_mask, Wq, bq, Wk, bk, Wv, bv, Wo, bo,
                  use_mask):
    in_maps = []
    for c in range(NCORES):
        b, g = divmod(c, NCORES // B)
        cols = slice(g * HPC * HD, (g + 1) * HPC * HD)
        wq_g = Wq[:, cols]
        wk_g = Wk[:, cols]
        qk_cols = [wq_g[:, 0:HD], wq_g[:, HD:2 * HD],
                   wk_g[:, 0:HD], wk_g[:, HD:2 * HD],
                   wq_g[:, 2 * HD:3 * HD], wk_g[:, 2 * HD:3 * HD]]
        wqk = np.concatenate(qk_cols, axis=1)
        bq_g = bq[cols]
        bk_g = bk[cols]
        bqk = np.concatenate([bq_g[0:HD], bq_g[HD:2 * HD],
                              bk_g[0:HD], bk_g[HD:2 * HD],
                              bq_g[2 * HD:3 * HD], bk_g[2 * HD:3 * HD]])
        xt = np.ascontiguousarray(hidden_states[b].T).astype(np.float32)
        xt8h, xt8l = _hilo(xt)
        wqk8h, wqk8l = _hilo(np.ascontiguousarray(wqk) * WS)
        wv8h, wv8l = _hilo(np.ascontiguousarray(Wv[:, cols]) * WS)
        m = {
            "xt8h": xt8h, "xt8l": xt8l,
            "wqk8h": wqk8h, "wqk8l": wqk8l,
            "wv8h": wv8h, "wv8l": wv8l,
            "wo": np.ascontiguousarray(Wo[cols, :]).astype(NP_CDT),
            "bqk": bqk.astype(np.float32).reshape(-1, 1),
        }
        if use_mask:
            mvec = np.exp(-10000.0 * (1.0 - attention_mask[b].astype(np.float64)))
            m["mv"] = mvec.astype(np.float32).reshape(-1, 1)
        in_maps.append(m)
    return in_maps


def kernel(hidden_states, attention_mask, Wq, bq, Wk, bk, Wv, bv, Wo, bo):
    hidden_states = np.asarray(hidden_states, np.float32)
    attention_mask = np.asarray(attention_mask)
    Wq, bq = np.asarray(Wq, np.float32), np.asarray(bq, np.float32)
    Wk, bk = np.asarray(Wk, np.float32), np.asarray(bk, np.float32)
    Wv, bv = np.asarray(Wv, np.float32), np.asarray(bv, np.float32)
    Wo, bo = np.asarray(Wo, np.float32), np.asarray(bo, np.float32)

    use_mask = not bool(np.all(attention_mask == 1))
    nc = _get_nc(use_mask)
    in_maps = _shard_inputs(hidden_states, attention_mask,
                            Wq, bq, Wk, bk, Wv, bv, Wo, bo, use_mask)
    res = run_bass_kernel_spmd(nc, in_maps, core_ids=list(range(NCORES)))

    const_row = (bv.astype(np.float64) @ Wo.astype(np.float64)
                 + bo.astype(np.float64))
    out = np.zeros((B, S, H), np.float64)
    for c in range(NCORES):
        b = c // (NCORES // B)
        out[b] += res.results[c]["out"].astype(np.float64)
    out += const_row[None, None, :]
    return out.astype(np.float32)


if __name__ == "__main__":
    rng = np.random.default_rng(0)
    inputs = {
        "hidden_states": rng.standard_normal((B, S, H), np.float32),
        "attention_mask": np.ones((B, S), np.int32),
        "Wq": rng.standard_normal((H, H), np.float32) * 0.02,
        "bq": np.zeros(H, np.float32),
        "Wk": rng.standard_normal((H, H), np.float32) * 0.02,
        "bk": np.zeros(H, np.float32),
        "Wv": rng.standard_normal((H, H), np.float32) * 0.02,
        "bv": np.zeros(H, np.float32),
        "Wo": rng.standard_normal((H, H), np.float32) * 0.02,
        "bo": np.zeros(H, np.float32),
    }
    out = kernel(**inputs)
    print("out", out.shape, out.dtype)


# revision 59
# speedup vs baseline: 1.0308x; 1.0308x over previous
"""BERT multi-head self-attention on 8 Trainium2 NeuronCores.

Problem: B=2, S=2048, H=768, NH=12, HD=64 (fp32 reference).

Sharding (hardcoded): core c in 0..7 handles batch b=c//4 and head group
g=c%4 (heads 3g..3g+2).  Each core computes its 3 heads' attention plus the
partial output projection ctx_g @ Wo[rows of g]; the host sums the 4 partial
outputs per batch element and adds the (bv @ Wo + bo) constant row.

Device pipeline per core (matmuls on PE, exp on ACT, evictions on DVE):
  1. QKV projections as fp8e4 DoubleRow matmuls: x and the (host-side
     64x-scaled) weights are split hi/lo into two e4m3 planes; the three
     cross terms xh@wh + xl@wh + xh@wl accumulate 256-deep contractions per
     instruction (25% fewer PE cycles than bf16) at bf16-grade accuracy.
     The PSUM evictions rescale by 1/64 and add the bias.
  2. V natural [seq, head_dim] with a ones-augmented column per head, so the
     P@V_aug matmul yields both ctx^T and the softmax denominator.
  3. scoresT[k, q] = K^T.T @ Q^T per 128-row k-block; ACT computes
     exp(0.125*scores) straight out of PSUM.
  4. ctxT_aug[d+1, q] accumulates over k-blocks in PSUM; row 64 is the
     denominator.  Reciprocal, then a partition-broadcast of 1/denom (bounced
     through a DRAM scratch row) and a DVE multiply normalize ctx^T.
  5. out[q, hout] = ctxT_norm.T @ Wo_slice per 128-row q-block; evictions
     stay off the ACT engine so it runs nothing but the exp stream.

Scheduling: one flat software pipeline over all (head, q-half, k-block)
units in which scores/exp lead their P@V consumer.  Only a 5-unit projection
prefix runs before the first exp; the remaining QKV-projection work drains
one unit per iteration inside blocks 0-1, in deadline order, borrowing
scores-PSUM slots.  PSUM: 3 score buffers x 2 banks + ctx accumulator x 2.
"""

import os
import sys
import numpy as np

for _p in ("/opt/trn_rl_repo",):
    if _p not in sys.path and os.path.isdir(_p):
        sys.path.append(_p)

import ml_dtypes  # noqa: E402

from concourse import bacc  # noqa: E402
import concourse.mybir as mybir  # noqa: E402
import concourse.tile as tile  # noqa: E402
from concourse.bass_utils import run_bass_kernel_spmd  # noqa: E402

B, S, H = 2, 2048, 768
NH, HD = 12, 64
HPC = 3
NCORES = 8
P = 128
NKB = S // P
NQB = S // P
NHC = H // P
NCP = NHC // 2
QH = 1024
NQH = S // QH
F32 = mybir.dt.float32
F8 = mybir.dt.float8e4

CDT = mybir.dt.bfloat16
NP_CDT = ml_dtypes.bfloat16
NP_F8 = ml_dtypes.float8_e4m3

WS = 64.0
DR = mybir.MatmulPerfMode.DoubleRow


def _build_nc(use_mask: bool):
    import contextlib

    nc = bacc.Bacc("TRN2", target_bir_lowering=False)
    AF = mybir.ActivationFunctionType
    MULT, ADD = mybir.AluOpType.mult, mybir.AluOpType.add

    xt8h = nc.dram_tensor("xt8h", [H, S], F8, kind="ExternalInput")
    xt8l = nc.dram_tensor("xt8l", [H, S], F8, kind="ExternalInput")
    wqk8h = nc.dram_tensor("wqk8h", [H, 2 * HPC * HD], F8, kind="ExternalInput")
    wqk8l = nc.dram_tensor("wqk8l", [H, 2 * HPC * HD], F8, kind="ExternalInput")
    wv8h = nc.dram_tensor("wv8h", [H, HPC * HD], F8, kind="ExternalInput")
    wv8l = nc.dram_tensor("wv8l", [H, HPC * HD], F8, kind="ExternalInput")
    wo = nc.dram_tensor("wo", [HPC * HD, H], CDT, kind="ExternalInput")
    bqk = nc.dram_tensor("bqk", [2 * HPC * HD, 1], F32, kind="ExternalInput")
    if use_mask:
        mv = nc.dram_tensor("mv", [S, 1], F32, kind="ExternalInput")
    out = nc.dram_tensor("out", [S, H], CDT, kind="ExternalOutput")
    rspill = nc.dram_tensor("rspill", [HPC * NQH, QH], F32)

    with tile.TileContext(nc) as tc, contextlib.ExitStack() as ctx, \
            nc.allow_low_precision(reason="fp8/bf16 compute pipeline by design"):
        const = ctx.enter_context(tc.tile_pool(name="const", bufs=1))
        xt_pool = ctx.enter_context(tc.tile_pool(name="xt", bufs=1))
        w_pool = ctx.enter_context(tc.tile_pool(name="w", bufs=1))
        qkt_pool = ctx.enter_context(tc.tile_pool(name="qkt", bufs=1))
        v_pool = ctx.enter_context(tc.tile_pool(name="v", bufs=1))
        pt_pool = ctx.enter_context(tc.tile_pool(name="pt", bufs=6))
        ctxu_pool = ctx.enter_context(tc.tile_pool(name="ctxu", bufs=1))
        ctxn_pool = ctx.enter_context(tc.tile_pool(name="ctxn", bufs=1))
        out_sb_pool = ctx.enter_context(tc.tile_pool(name="outsb", bufs=6))

        bias_t = const.tile([P, 3], F32, tag="bqk")
        wqk8h_t = w_pool.tile([P, NHC, 2 * HPC * HD], F8, tag="wqk8h")
        wqk8l_t = w_pool.tile([P, NHC, 2 * HPC * HD], F8, tag="wqk8l")
        wv8h_t = w_pool.tile([P, NHC, HPC * HD], F8, tag="wv8h")
        wv8l_t = w_pool.tile([P, NHC, HPC * HD], F8, tag="wv8l")
        nc.sync.dma_start(
            wqk8h_t[:], wqk8h[:].rearrange("(c p) n -> p c n", p=P))
        nc.scalar.dma_start(
            wqk8l_t[:], wqk8l[:].rearrange("(c p) n -> p c n", p=P))
        xt8h_p = [xt_pool.tile([P, 2, S], F8, tag=f"xt8h{cp}",
                               name=f"xt8h{cp}") for cp in range(NCP)]
        xt8l_p = [xt_pool.tile([P, 2, S], F8, tag=f"xt8l{cp}",
                               name=f"xt8l{cp}") for cp in range(NCP)]

        def xt_piece(eng, dst_p, src, cp, qs):
            eng.dma_start(
                dst_p[cp][:, :, qs],
                src[2 * cp * P:(2 * cp + 2) * P, qs].rearrange(
                    "(c p) n -> p c n", p=P))

        for cp in range(NCP):
            xt_piece(nc.sync, xt8h_p, xt8h, cp, slice(0, QH))
            xt_piece(nc.scalar, xt8l_p, xt8l, cp, slice(0, QH))
        nc.sync.dma_start(
            bias_t[:], bqk[:].rearrange("(m p) one -> p (m one)", p=P))
        nc.scalar.dma_start(
            wv8h_t[:], wv8h[:].rearrange("(c p) n -> p c n", p=P))
        nc.scalar.dma_start(
            wv8l_t[:], wv8l[:].rearrange("(c p) n -> p c n", p=P))
        for cp in range(NCP):
            xt_piece(nc.sync, xt8h_p, xt8h, cp, slice(QH, S))
            xt_piece(nc.scalar, xt8l_p, xt8l, cp, slice(QH, S))
        bias_sb = [bias_t[:, m:m + 1] for m in range(3)]
        if use_mask:
            mv_t = const.tile([P, NKB], F32, tag="mv")
            nc.scalar.dma_start(
                mv_t[:], mv[:].rearrange("(kb p) one -> p (kb one)", p=P))
            mv_sb = [mv_t[:, kb:kb + 1] for kb in range(NKB)]
        wo_t = w_pool.tile([HD, HPC, H], CDT, tag="wo")
        nc.scalar.dma_start(
            wo_t[:], wo[:].rearrange("(h p) n -> p h n", p=HD))
        wo_sb = [wo_t[:, h, :] for h in range(HPC)]

        tq01 = qkt_pool.tile([P, S], CDT, tag="tq01")
        tk01 = qkt_pool.tile([P, S], CDT, tag="tk01")
        tqk2 = qkt_pool.tile([P, S], CDT, tag="tqk2")
        qkt_tiles = [tq01, tk01, tqk2]
        v_sb = [None] * NKB

        HILO = ((xt8h_p, wqk8h_t, wv8h_t), (xt8l_p, wqk8h_t, wv8h_t),
                (xt8h_p, wqk8l_t, wv8l_t))

        def qkt_unit(psum_tile_fn, qc, m):
            qs = slice(qc * 512, (qc + 1) * 512)
            ms = slice(m * P, (m + 1) * P)
            ps = psum_tile_fn([P, 512], "qkps")
            for t, (xa, wa, _) in enumerate(HILO):
                for cp in range(NCP):
                    cs = slice(2 * cp, 2 * cp + 2)
                    nc.tensor.matmul(
                        ps[:], wa[:, cs, ms], xa[cp][:, :, qs],
                        start=(t == 0 and cp == 0),
                        stop=(t == 2 and cp == NCP - 1),
                        perf_mode=DR,
                    )
            nc.vector.tensor_scalar(
                qkt_tiles[m][:, qs], ps[:], 1.0 / WS, bias_sb[m][:],
                MULT, ADD,
            )

        def v_unit(psum_tile_fn, kb):
            ks = slice(kb * P, (kb + 1) * P)
            ps = psum_tile_fn([P, HPC * HD], "vps")
            for t, (xa, _, va) in enumerate(HILO):
                for cp in range(NCP):
                    cs = slice(2 * cp, 2 * cp + 2)
                    nc.tensor.matmul(
                        ps[:], xa[cp][:, :, ks], va[:, cs, :],
                        start=(t == 0 and cp == 0),
                        stop=(t == 2 and cp == NCP - 1),
                        perf_mode=DR,
                    )
            vt = v_pool.tile([P, HPC, HD + 1], CDT, tag=f"v{kb}",
                             name=f"vt{kb}")
            nc.vector.tensor_scalar_mul(
                vt[:, :, 0:HD], ps[:].rearrange("p (h d) -> p h d", h=HPC),
                1.0 / WS,
            )
            nc.vector.memset(vt[:, :, HD:HD + 1], 1.0)
            if use_mask:
                nc.vector.tensor_scalar_mul(vt[:], vt[:], mv_sb[kb][:])
            v_sb[kb] = vt

        with tc.tile_pool(name="qkt_psum", bufs=2, space="PSUM") as qkt_psum:
            def pre_tile(shape, name):
                return qkt_psum.tile(shape, F32, tag="qkt", name=name)
            qkt_unit(pre_tile, 0, 0)
            qkt_unit(pre_tile, 0, 1)
            qkt_unit(pre_tile, 1, 0)
            v_unit(pre_tile, 0)
        tk2 = qkt_pool.tile([HD, S], CDT, tag="tk2")

        def q_ap(h, sl):
            if h == 0:
                return tq01[0:HD, sl]
            if h == 1:
                return tq01[HD:2 * HD, sl]
            return tqk2[0:HD, sl]

        def k_ap(h, sl):
            if h == 0:
                return tk01[0:HD, sl]
            if h == 1:
                return tk01[HD:2 * HD, sl]
            return tk2[0:HD, sl]

        ctxu_t = [ctxu_pool.tile([HD, S], F32, tag=f"ctxu{h}", name=f"ctxu{h}")
                  for h in range(HPC)]
        ctxn_t = [ctxn_pool.tile([HD, S], CDT, tag=f"ctxn{h}", name=f"ctxn{h}")
                  for h in range(HPC)]
        recip_t = [ctxu_pool.tile([65, S], F32, tag=f"recip{h}",
                                  name=f"recip{h}") for h in range(HPC)]
        rbc_pool = ctx.enter_context(tc.tile_pool(name="rbc", bufs=2))

        def op_unit(psum_tile_fn, qb):
            qsl = slice(qb * P, (qb + 1) * P)
            ops = psum_tile_fn([P, H], "ops")
            for nchunk in range(2):
                nsl = slice(nchunk * 512, min((nchunk + 1) * 512, H))
                for h in range(HPC):
                    nc.tensor.matmul(
                        ops[:, nsl],
                        ctxn_t[h][:, qsl],
                        wo_sb[h][:, nsl],
                        start=(h == 0), stop=(h == HPC - 1),
                    )
            osb = out_sb_pool.tile([P, H], CDT, tag="osb", name="osb")
            # the output tail runs after the exp stream, so ACT is free to
            # take half the evictions; DMAs alternate the sync/ACT rings
            if qb % 2 == 0:
                nc.vector.tensor_copy(osb[:], ops[:])
                nc.sync.dma_start(out[qsl, :], osb[:])
            else:
                nc.scalar.copy(osb[:], ops[:])
                nc.scalar.dma_start(out[qsl, :], osb[:])

        with tc.tile_pool(name="sc_psum", bufs=3, space="PSUM") as sc_psum, \
             tc.tile_pool(name="ctx_psum", bufs=1, space="PSUM") as ctx_psum:
            def sc_tile(shape, name):
                return sc_psum.tile(shape, F32, tag="sc", name=name)

            sched = ([("qk", 1, 1)]
                     + [("v", kb) for kb in (1, 2, 3, 4, 5, 6)] + [("qk", 2, 1)]
                     + [("v", kb) for kb in (7, 8, 9, 10)] + [("qk", 3, 1)]
                     + [("v", kb) for kb in (11, 12, 13, 14, 15)]
                     + [("qk", 2, 2), ("qk", 3, 2), ("qk", 0, 2),
                        ("qk", 1, 2), ("qk", 2, 0), ("qk", 3, 0)])
            fillers = []
            for u in sched:
                if u[0] == "v":
                    fillers.append(lambda kb=u[1]: v_unit(sc_tile, kb))
                else:
                    fillers.append(
                        lambda qc=u[1], m=u[2]: qkt_unit(sc_tile, qc, m))
            fillers.reverse()

            blocks = [(qh, h) for qh in range(NQH) for h in range(HPC)]

            def scores(bi, kb):
                qh, h = blocks[bi]
                ksl = slice(kb * P, (kb + 1) * P)
                sps = sc_psum.tile([P, QH], F32, tag="sc", name="sps")
                for c in range(QH // 512):
                    nc.tensor.matmul(
                        sps[:, c * 512:(c + 1) * 512],
                        k_ap(h, ksl),
                        q_ap(h, slice(qh * QH + c * 512,
                                      qh * QH + (c + 1) * 512)),
                        start=True, stop=True,
                    )
                pt = pt_pool.tile([P, QH], CDT, tag="pt", name="pt")
                nc.scalar.activation(pt[:], sps[:], AF.Exp, scale=0.125)
                return pt

            def pv(bi, kb, pt, cps):
                _, h = blocks[bi]
                for c in range(QH // 512):
                    nc.tensor.matmul(
                        cps[:, c * 512:(c + 1) * 512],
                        v_sb[kb][:, h, :],
                        pt[:, c * 512:(c + 1) * 512],
                        start=(kb == 0), stop=(kb == NKB - 1),
                    )

            def normalize(bi, cps):
                qh, h = blocks[bi]
                qsl = slice(qh * QH, (qh + 1) * QH)
                nc.vector.tensor_copy(ctxu_t[h][:, qsl], cps[0:HD, :])
                nc.vector.reciprocal(
                    recip_t[h][HD:HD + 1, qsl], cps[HD:HD + 1, :]
                )
                row = qh * HPC + h
                nc.sync.dma_start(rspill[row, :], recip_t[h][HD:HD + 1, qsl])
                rbc = rbc_pool.tile([HD, QH], F32, tag="rbc", name="rbc")
                nc.sync.dma_start(
                    rbc[:], rspill[row:row + 1, :].to_broadcast((HD, QH)))
                nc.vector.tensor_mul(ctxn_t[h][:, qsl], ctxu_t[h][:, qsl],
                                     rbc[:])

            from collections import deque
            cps_of = {}
            pending = deque()
            gi = 0

            def drain_one():
                pbi, pkb, ppt = pending.popleft()
                pv(pbi, pkb, ppt, cps_of[pbi])
                if pkb == NKB - 1:
                    normalize(pbi, cps_of[pbi])

            for bi in range(len(blocks)):
                for kb in range(NKB):
                    if kb == 0:
                        cps_of[bi] = ctx_psum.tile(
                            [HD + 1, QH], F32, tag="ctx", name=f"cps{bi}")
                        if bi == 2:
                            nc.sync.dma_start(tk2[:], tqk2[HD:2 * HD, :])
                    pending.append((bi, kb, scores(bi, kb)))
                    gi += 1
                    for _ in range(2 if gi <= 2 else 1):
                        if fillers:
                            fillers.pop()()
                    lag = 1 if fillers else 3
                    while len(pending) > lag:
                        drain_one()
            while pending:
                drain_one()

        with tc.tile_pool(name="op_psum", bufs=4, space="PSUM") as op_psum:
            def op_tile(shape, name):
                return op_psum.tile(shape, F32, tag="op", name=name)
            for qb in range(NQB):
                op_unit(op_tile, qb)

    nc.compile()
    return nc


_NC_CACHE = {}


def _get_nc(use_mask: bool):
    if use_mask not in _NC_CACHE:
        _NC_CACHE[use_mask] = _build_nc(use_mask)
    return _NC_CACHE[use_mask]


def _hilo(a):
    hi = a.astype(NP_F8)
    lo = (a - hi.astype(np.float32)).astype(NP_F8)
    return hi, lo


def _shard_inputs(hidden_states, attention_mask, Wq, bq, Wk, bk, Wv, bv, Wo, bo,
                  use_mask):
    in_maps = []
    for c in range(NCORES):
        b, g = divmod(c, NCORES // B)
        cols = slice(g * HPC * HD, (g + 1) * HPC * HD)
        wq_g = Wq[:, cols]
        wk_g = Wk[:, cols]
        qk_cols = [wq_g[:, 0:HD], wq_g[:, HD:2 * HD],
                   wk_g[:, 0:HD], wk_g[:, HD:2 * HD],
                   wq_g[:, 2 * HD:3 * HD], wk_g[:, 2 * HD:3 * HD]]
        wqk = np.concatenate(qk_cols, axis=1)
        bq_g = bq[cols]
        bk_g = bk[cols]
        bqk = np.concatenate([bq_g[0:HD], bq_g[HD:2 * HD],
                              bk_g[0:HD], bk_g[HD:2 * HD],
                              bq_g[2 * HD:3 * HD], bk_g[2 * HD:3 * HD]])
        xt = np.ascontiguousarray(hidden_states[b].T).astype(np.float32)
        xt8h, xt8l = _hilo(xt)
        wqk8h, wqk8l = _hilo(np.ascontiguousarray(wqk) * WS)
        wv8h, wv8l = _hilo(np.ascontiguousarray(Wv[:, cols]) * WS)
        m = {
            "xt8h": xt8h, "xt8l": xt8l,
            "wqk8h": wqk8h, "wqk8l": wqk8l,
            "wv8h": wv8h, "wv8l": wv8l,
            "wo": np.ascontiguousarray(Wo[cols, :]).astype(NP_CDT),
            "bqk": bqk.astype(np.float32).reshape(-1, 1),
        }
        if use_mask:
            mvec = np.exp(-10000.0 * (1.0 - attention_mask[b].astype(np.float64)))
            m["mv"] = mvec.astype(np.float32).reshape(-1, 1)
        in_maps.append(m)
    return in_maps


def kernel(hidden_states, attention_mask, Wq, bq, Wk, bk, Wv, bv, Wo, bo):
    hidden_states = np.asarray(hidden_states, np.float32)
    attention_mask = np.asarray(attention_mask)
    Wq, bq = np.asarray(Wq, np.float32), np.asarray(bq, np.float32)
    Wk, bk = np.asarray(Wk, np.float32), np.asarray(bk, np.float32)
    Wv, bv = np.asarray(Wv, np.float32), np.asarray(bv, np.float32)
    Wo, bo = np.asarray(Wo, np.float32), np.asarray(bo, np.float32)

    use_mask = not bool(np.all(attention_mask == 1))
    nc = _get_nc(use_mask)
    in_maps = _shard_inputs(hidden_states, attention_mask,
                            Wq, bq, Wk, bk, Wv, bv, Wo, bo, use_mask)
    res = run_bass_kernel_spmd(nc, in_maps, core_ids=list(range(NCORES)))

    const_row = (bv.astype(np.float64) @ Wo.astype(np.float64)
                 + bo.astype(np.float64))
    out = np.zeros((B, S, H), np.float64)
    for c in range(NCORES):
        b = c // (NCORES // B)
        out[b] += res.results[c]["out"].astype(np.float64)
    out += const_row[None, None, :]
    return out.astype(np.float32)


if __name__ == "__main__":
    rng = np.random.default_rng(0)
    inputs = {
        "hidden_states": rng.standard_normal((B, S, H), np.float32),
        "attention_mask": np.ones((B, S), np.int32),
        "Wq": rng.standard_normal((H, H), np.float32) * 0.02,
        "bq": np.zeros(H, np.float32),
        "Wk": rng.standard_normal((H, H), np.float32) * 0.02,
        "bk": np.zeros(H, np.float32),
        "Wv": rng.standard_normal((H, H), np.float32) * 0.02,
        "bv": np.zeros(H, np.float32),
        "Wo": rng.standard_normal((H, H), np.float32) * 0.02,
        "bo": np.zeros(H, np.float32),
    }
    out = kernel(**inputs)
    print("out", out.shape, out.dtype)
